# revision 37
# baseline (speedup 1.0000x reference)
"""Trainium2 Bass kernel for nn_MultiHeadAttention (B=2,S=128,H=16,W=16,E=256, 8 heads).

Sharding: the 512 independent (b,h,w) slices are split 64-per-core across 8
NeuronCores (pure SPMD, no collectives). Each slice is a [S=128, E=256]
self-attention problem.

v4 design (per core, per group of 4 slices):
  - in_proj Q^T/K^T as [f,t] matmuls (weights stationary); Q evicted as
    4-way block-diag qb4 (rmask multiply), K plain; V in natural [t,c]
    layout evicted into ones-augmented v33 tiles [t, (sl, j, c|1)] whose
    every 33rd column is a persistent 1.0 (memset once per buffer).
  - scores PRE-TRANSPOSED S^T[k,(j,q)]: causal -1000 mask added by a
    rank-128 const matmul (start of group), scores accumulate on top;
    exp on ACT gives exact zeros on masked entries.
  - P@V + rowsum FUSED: lhsT = pn block (P as stationary weights),
    rhs = [V_j | ones] 33 cols -> po2[q, (j, c|Z)] per slice. No separate
    zsum matmuls, no indicator broadcasts.
  - normalize: Z cols evicted, one reciprocal [128,32] per group, per-slice
    DVE multiply (po2 PSUM x broadcast recip) -> on[q, (hg, sl, j, c)] bf16.
  - one blocked DMA transpose per group -> on_t[(j,c), (hg, sl), q];
    out_proj bf16 (weights stationary), y evicted bf16, DRAM via SWDGE.
"""

import os
import sys

import numpy as np

sys.path.insert(0, "/opt/trn_rl_repo")

from contextlib import ExitStack

import concourse.bass as bass
import concourse.mybir as mybir
import concourse.tile as tile
from concourse import bacc
from concourse.bass_utils import run_bass_kernel_spmd

P = 128
NCORES = 8
NSLICE = 64  # slices per core
GSL = 4  # slices per group
NG = NSLICE // GSL  # groups per core
NH = 8
HD = 32
E = 256
S = 128

F32 = mybir.dt.float32
BF16 = mybir.dt.bfloat16
AX = mybir.AxisListType
ALU = mybir.AluOpType
AF = mybir.ActivationFunctionType


def build_program(ng=NG, repeats=1):
    nc = bacc.Bacc("TRN2", target_bir_lowering=False, debug=False, num_devices=NCORES)

    x_d = nc.dram_tensor("x", [ng, P, 2, GSL * S], BF16, kind="ExternalInput").ap()
    wq_d = nc.dram_tensor("wq", [P, 2, 256], BF16, kind="ExternalInput").ap()
    wk_d = nc.dram_tensor("wk", [P, 2, 256], BF16, kind="ExternalInput").ap()
    wv_d = nc.dram_tensor("wv", [P, 2, 256], BF16, kind="ExternalInput").ap()
    wo_d = nc.dram_tensor("wo", [P, 2, 256], BF16, kind="ExternalInput").ap()
    rm_d = nc.dram_tensor("rm2", [P, 4], F32, kind="ExternalInput").ap()
    am_d = nc.dram_tensor("amask", [P, S], BF16, kind="ExternalInput").ap()
    ni_d = nc.dram_tensor("negi", [P, 4 * S], BF16, kind="ExternalInput").ap()
    y_d = nc.dram_tensor("y", [ng, 2, P, GSL * S], BF16, kind="ExternalOutput").ap()

    with tile.TileContext(nc) as tc, ExitStack() as ctx:
        const = ctx.enter_context(tc.tile_pool(name="const", bufs=1))
        wq = const.tile([P, 2, 256], BF16, tag="wq")
        wk = const.tile([P, 2, 256], BF16, tag="wk")
        wv = const.tile([P, 2, 256], BF16, tag="wv")
        wo = const.tile([P, 2, 256], BF16, tag="wo")
        rm2 = const.tile([P, 4], F32, tag="rm2")
        amask = const.tile([P, S], BF16, tag="amask")
        negi = const.tile([P, 4 * S], BF16, tag="negi")
        v33a = const.tile([P, GSL, NH, 33], BF16, tag="v33a")
        v33b = const.tile([P, GSL, NH, 33], BF16, tag="v33b")
        nc.sync.dma_start(wq[:], wq_d)
        nc.sync.dma_start(wk[:], wk_d)
        nc.sync.dma_start(wv[:], wv_d)
        nc.sync.dma_start(wo[:], wo_d)
        nc.sync.dma_start(rm2[:], rm_d)
        nc.sync.dma_start(amask[:], am_d)
        nc.sync.dma_start(negi[:], ni_d)
        nc.vector.memset(v33a[:, :, :, 32], 1.0)
        nc.vector.memset(v33b[:, :, :, 32], 1.0)
        v33s = [v33a, v33b]

        xp = ctx.enter_context(tc.tile_pool(name="xp", bufs=3))
        qtp = ctx.enter_context(tc.tile_pool(name="qtp", bufs=2))
        ktp = ctx.enter_context(tc.tile_pool(name="ktp", bufs=2))
        pnp = ctx.enter_context(tc.tile_pool(name="pnp", bufs=4))
        rcpp = ctx.enter_context(tc.tile_pool(name="rcpp", bufs=2))
        onp = ctx.enter_context(tc.tile_pool(name="onp", bufs=2))
        otp = ctx.enter_context(tc.tile_pool(name="otp", bufs=2))
        ysp = ctx.enter_context(tc.tile_pool(name="ysp", bufs=2))

        # PSUM: 2 (scores) + 2 (in_proj + py) + 4 (po2) = 8 banks
        psS = ctx.enter_context(tc.tile_pool(name="psS", bufs=2, space="PSUM"))
        psD = ctx.enter_context(tc.tile_pool(name="psD", bufs=2, space="PSUM"))
        psO = ctx.enter_context(tc.tile_pool(name="psO", bufs=4, space="PSUM"))

        def emit_x_load(g):
            x = xp.tile([P, 2, GSL, S], BF16, tag="x")
            nc.sync.dma_start(x[:], x_d[g].rearrange("p c (sl s) -> p c sl s", sl=GSL))
            return x

        def emit_in_proj(x, g):
            """kt: [f', (hg, sl, t)] bf16; qb4: block-diag Q [f', (hg, u, sl, t)]
            bf16; v33: [t, (sl, j, c|1)] bf16 (ones col persists)."""
            qb4 = qtp.tile([P, 2, 4, GSL, S], BF16, tag="qb4")
            kt = ktp.tile([P, 2, GSL, S], BF16, tag="kt")
            v33 = v33s[g % 2]
            for which, wmat in ((0, wq), (1, wk)):
                for ft in range(2):
                    ps = psD.tile([P, GSL * S], F32, tag="d")
                    for ec in range(2):
                        nc.tensor.matmul(
                            ps[:],
                            lhsT=wmat[:, ec, ft * P : (ft + 1) * P],
                            rhs=x[:, ec].rearrange("p a b -> p (a b)"),
                            start=(ec == 0),
                            stop=(ec == 1),
                        )
                    if which == 0:
                        nc.vector.tensor_tensor(
                            qb4[:, ft].rearrange("p u a b -> p u (a b)"),
                            ps[:].rearrange("p (u f) -> p u f", u=1).broadcast_to(
                                [P, 4, GSL * S]
                            ),
                            rm2[:, :].rearrange("p (u f) -> p u f", f=1).broadcast_to(
                                [P, 4, GSL * S]
                            ),
                            ALU.mult,
                        )
                    else:
                        nc.scalar.copy(
                            kt[:, ft].rearrange("p a b -> p (a b)"), ps[:]
                        )
            for slp in range(GSL // 2):
                psv = psD.tile([P, 2, 256], F32, tag="d")
                for half in range(2):
                    sl = slp * 2 + half
                    for ec in range(2):
                        nc.tensor.matmul(
                            psv[:, half],
                            lhsT=x[:, ec, sl, :],
                            rhs=wv[:, ec, :],
                            start=(ec == 0),
                            stop=(ec == 1),
                        )
                dvv = v33[:, slp * 2 : slp * 2 + 2, :, 0:32]
                src = psv[:].rearrange("p a (b c) -> p a b c", b=NH)
                if slp == 0:
                    nc.scalar.copy(dvv, src)
                else:
                    nc.vector.tensor_copy(dvv, src)
            return qb4, kt, v33

        def emit_scores_exp(kqv, sl):
            """S^T[k,(j,q)] with causal -1000 via const mask matmul; exp on ACT
            (masked entries underflow to exact 0)."""
            qb4, kt, v33 = kqv
            pn = pnp.tile([P, 2, GSL, S], BF16, tag="pn")
            for hg in range(2):
                ssc = psS.tile([P, GSL * S], F32, tag="s")
                nc.tensor.matmul(
                    ssc[:],
                    lhsT=amask[:],
                    rhs=negi[:],
                    start=True,
                    stop=False,
                    skip_group_check=True,
                )
                nc.tensor.matmul(
                    ssc[:],
                    lhsT=kt[:, hg, sl, :],
                    rhs=qb4[:, hg, :, sl, :],
                    start=False,
                    stop=True,
                    skip_group_check=True,
                )
                nc.scalar.activation(
                    pn[:, hg].rearrange("p a b -> p (a b)"), ssc[:], AF.Exp
                )
            return pn

        def emit_pv2(kqv, pn, sl, zg):
            """po2[q, (j, c|Z)]: P stationary, rhs = [V_j | 1] (33 cols);
            Z column evicted into zg[:, sl]."""
            v33 = kqv[2]
            po2 = psO.tile([P, NH, 33], F32, tag="po2")
            for hg in range(2):
                for jp in range(4):
                    j = hg * 4 + jp
                    nc.tensor.matmul(
                        po2[:, j, :],
                        lhsT=pn[:, hg, jp, :],
                        rhs=v33[:, sl, j, :],
                        skip_group_check=True,
                    )
            nc.vector.tensor_copy(zg[:, sl], po2[:, :, 32])
            return po2

        def emit_norm_sl(po2, rcpz, on, sl):
            """on[q, (hg, sl, j, c)] = po2[q, (j, c)] * (1/Z) (broadcast)."""
            nc.vector.tensor_tensor(
                on[:, :, sl],
                po2[:, :, 0:32].rearrange("p (hg jp) c -> p hg jp c", hg=2),
                rcpz[:, sl].broadcast_to([P, 2, 4, 32]),
                ALU.mult,
            )

        def emit_out_proj(on_t, g):
            y_sb = ysp.tile([P, 2, GSL * S], BF16, tag="ysb")
            for et in range(2):
                py = psD.tile([P, GSL * S], F32, tag="d")
                for cc in range(2):
                    nc.tensor.matmul(
                        py[:],
                        lhsT=wo[:, cc, et * P : (et + 1) * P],
                        rhs=on_t[:, cc * GSL : (cc + 1) * GSL, :],
                        start=(cc == 0),
                        stop=(cc == 1),
                    )
                if et == 0:
                    nc.scalar.copy(y_sb[:, et], py[:])
                else:
                    nc.vector.tensor_copy(y_sb[:, et], py[:])
            nc.gpsimd.dma_start(y_d[g].rearrange("e p f -> p e f"), y_sb[:])

        def emit_group_tail(zg, po2s, g):
            rcpz = rcpp.tile([P, GSL, 2, 4, 1], F32, tag="rcpz")
            with nc.allow_low_precision(reason="softmax denominators"):
                nc.vector.reciprocal(
                    rcpz[:].rearrange("p a b c d -> p (a b c d)"),
                    zg[:].rearrange("p a b -> p (a b)"),
                )
            on = onp.tile([P, 2, GSL, 4, 32], BF16, tag="on")
            for sl in range(GSL):
                emit_norm_sl(po2s[sl], rcpz, on, sl)
            on_t = otp.tile([P, 2 * GSL, S], BF16, tag="ont")
            nc.sync.dma_start_transpose(
                on_t[:], on[:].rearrange("p a b c d -> p (a b c d)")
            )
            return on_t

        for _rep in range(repeats):
            xs = {}
            kqvs = {}
            pns = {}
            pending = None
            xs[0] = emit_x_load(0)
            kqvs[0] = emit_in_proj(xs[0], 0)
            for g in range(ng):
                kqv = kqvs[g]
                zg = rcpp.tile([P, GSL, NH], F32, tag="zg", name="zg")
                po2s = {}
                if g + 1 < ng:
                    xs[g + 1] = emit_x_load(g + 1)
                if (g, 0) not in pns:
                    pns[(g, 0)] = emit_scores_exp(kqv, 0)
                pns[(g, 1)] = emit_scores_exp(kqv, 1)
                po2s[0] = emit_pv2(kqv, pns[(g, 0)], 0, zg)
                if pending is not None:
                    emit_out_proj(*pending)
                    pending = None
                if g + 1 < ng:
                    kqvs[g + 1] = emit_in_proj(xs[g + 1], g + 1)
                pns[(g, 2)] = emit_scores_exp(kqv, 2)
                po2s[1] = emit_pv2(kqv, pns[(g, 1)], 1, zg)
                pns[(g, 3)] = emit_scores_exp(kqv, 3)
                po2s[2] = emit_pv2(kqv, pns[(g, 2)], 2, zg)
                po2s[3] = emit_pv2(kqv, pns[(g, 3)], 3, zg)
                if g + 1 < ng:
                    pns[(g + 1, 0)] = emit_scores_exp(kqvs[g + 1], 0)
                on_t = emit_group_tail(zg, po2s, g)
                pending = (on_t, g)
                for sl in range(GSL):
                    del pns[(g, sl)]
                del kqvs[g], xs[g]
            emit_out_proj(*pending)

    nc.compile()
    return nc


def prep_inputs(hidden_state, w_in, w_out):
    """Host-side prep: permute weights per-head, transpose x, shard."""
    import ml_dtypes

    bf16 = ml_dtypes.bfloat16
    B, S_, H, W, E_ = hidden_state.shape
    nsl = B * H * W
    scale = 1.0 / np.sqrt(HD)

    idx_q = np.concatenate([np.arange(i * 96, i * 96 + 32) for i in range(NH)])
    idx_k = idx_q + 32
    idx_v = idx_q + 64
    Wq = (w_in[idx_q] * scale).astype(np.float32)  # [256 f, 256 e]
    Wk = w_in[idx_k].astype(np.float32)
    Wv = w_in[idx_v].astype(np.float32)

    def pack_w(Wm):
        # [p, ec, f] with w[p, ec, f] = Wm[f, ec*128+p]
        return np.ascontiguousarray(
            Wm.T.reshape(2, P, 256).transpose(1, 0, 2)
        ).astype(bf16)

    wq_h = pack_w(Wq)
    wk_h = pack_w(Wk)
    wv_h = pack_w(Wv)
    wo_h = pack_w(w_out.astype(np.float32))

    # rm2[p, u] = 1 iff p//32 == u (block-diag eviction masks)
    rm2_h = np.zeros((P, 4), np.float32)
    for p in range(P):
        rm2_h[p, p // 32] = 1.0

    # mask matmul constants (S^T orientation): out[k,(j,q)] = -1000*am[q,k]
    #   want -1000 iff k > q  ->  am = triu(ones, +1)
    am_h = np.triu(np.ones((S, S), np.float32), 1).astype(bf16)
    ni_h = np.ascontiguousarray(
        (-1000.0 * np.eye(S, dtype=np.float32))[:, None, :].repeat(4, 1).reshape(S, 4 * S)
    ).astype(bf16)

    # x^T per slice: [slice, e, s]
    xt = hidden_state.transpose(0, 2, 3, 4, 1).reshape(nsl, E_, S_).astype(bf16)

    in_maps = []
    for c in range(NCORES):
        xs = xt[c * NSLICE : (c + 1) * NSLICE]  # [64, 256, 128]
        xs = xs.reshape(NG, GSL, 2, P, S_).transpose(0, 3, 2, 1, 4)
        xs = np.ascontiguousarray(xs.reshape(NG, P, 2, GSL * S_))
        in_maps.append(
            {
                "x": xs,
                "wq": wq_h,
                "wk": wk_h,
                "wv": wv_h,
                "wo": wo_h,
                "rm2": rm2_h,
                "amask": am_h,
                "negi": ni_h,
            }
        )
    return in_maps


def assemble_output(results, B=2, H=16, W=16):
    """results: list of 8 dicts with 'y' [NG, 2, 128, GSL*S] bf16."""
    ys = []
    for c in range(NCORES):
        y = np.asarray(results[c]["y"], dtype=np.float32)
        y = y.reshape(NG, 2, P, GSL, S)
        y = y.transpose(0, 3, 1, 2, 4).reshape(NSLICE, E, S)
        ys.append(y)
    y_all = np.concatenate(ys, axis=0)  # [512, 256 e, 128 s]
    y_all = y_all.transpose(0, 2, 1)  # [512, s, e]
    out = y_all.reshape(B, H, W, S, E).transpose(0, 3, 1, 2, 4)
    return np.ascontiguousarray(out.astype(np.float32))


_NC_CACHE = {}


def get_program(repeats=1):
    key = repeats
    if key not in _NC_CACHE:
        _NC_CACHE[key] = build_program(repeats=repeats)
    return _NC_CACHE[key]


class _Executor:
    """Cached PJRT executor: builds the shard_map jit once, reuses across calls."""

    def __init__(self, nc):
        import jax
        from jax.sharding import Mesh, PartitionSpec
        from jax.experimental.shard_map import shard_map
        from concourse.bass2jax import _bass_exec_p, install_neuronx_cc_hook, partition_id_tensor

        install_neuronx_cc_hook()
        self.nc = nc
        pname = nc.partition_id_tensor.name if nc.partition_id_tensor else None
        in_names, out_names, out_avals, zero_outs = [], [], [], []
        for alloc in nc.m.functions[0].allocations:
            if not isinstance(alloc, mybir.MemoryLocationSet):
                continue
            name = alloc.memorylocations[0].name
            if alloc.kind == "ExternalInput":
                if name != pname:
                    in_names.append(name)
            elif alloc.kind == "ExternalOutput":
                out_names.append(name)
                shape = tuple(alloc.tensor_shape)
                dtype = mybir.dt.np(alloc.dtype)
                out_avals.append(jax.core.ShapedArray(shape, dtype))
                zero_outs.append(np.zeros(shape, dtype))
        self.in_names = in_names
        self.out_names = out_names
        self.out_avals = out_avals
        n_params = len(in_names)
        all_names = in_names + out_names + ([pname] if pname else [])

        def _body(*args):
            operands = list(args)
            if pname is not None:
                operands.append(partition_id_tensor())
            return tuple(
                _bass_exec_p.bind(
                    *operands,
                    out_avals=tuple(out_avals),
                    in_names=tuple(all_names),
                    out_names=tuple(out_names),
                    lowering_input_output_aliases=(),
                    sim_require_finite=True,
                    sim_require_nnan=True,
                    nc=nc,
                )
            )

        devices = jax.devices()[:NCORES]
        mesh = Mesh(np.asarray(devices), ("core",))
        n_outs = len(out_avals)
        self._jit = jax.jit(
            shard_map(
                _body,
                mesh=mesh,
                in_specs=(PartitionSpec("core"),) * (n_params + n_outs),
                out_specs=(PartitionSpec("core"),) * n_outs,
                check_rep=False,
            ),
            keep_unused=True,
        )
        self._zero_concat = [
            np.zeros((NCORES * z.shape[0], *z.shape[1:]), z.dtype) for z in zero_outs
        ]
        self._jax = jax

    def run(self, in_maps):
        concat_in = [
            np.concatenate([np.asarray(in_maps[c][nm]) for c in range(NCORES)], axis=0)
            for nm in self.in_names
        ]
        outs = self._jit(*concat_in, *self._zero_concat)
        self._jax.block_until_ready(outs)
        return [
            {
                nm: np.asarray(outs[i]).reshape(NCORES, *self.out_avals[i].shape)[c]
                for i, nm in enumerate(self.out_names)
            }
            for c in range(NCORES)
        ]


_EXEC_CACHE = {}


def get_executor(repeats=1):
    if repeats not in _EXEC_CACHE:
        _EXEC_CACHE[repeats] = _Executor(get_program(repeats))
    return _EXEC_CACHE[repeats]


def kernel(hidden_state, w_in, w_out, repeats=1):
    hidden_state = np.asarray(hidden_state, dtype=np.float32)
    w_in = np.asarray(w_in, dtype=np.float32)
    w_out = np.asarray(w_out, dtype=np.float32)
    ex = get_executor(repeats)
    in_maps = prep_inputs(hidden_state, w_in, w_out)
    results = ex.run(in_maps)
    return assemble_output(results)


# revision 40
# speedup vs baseline: 1.2938x; 1.2938x over previous
"""Trainium2 Bass kernel for nn_MultiHeadAttention (B=2,S=128,H=16,W=16,E=256, 8 heads).

Sharding: the 512 independent (b,h,w) slices are split 64-per-core across 8
NeuronCores (pure SPMD, no collectives). Each slice is a [S=128, E=256]
self-attention problem.

v4 design (per core, per group of 4 slices):
  - in_proj Q^T/K^T as [f,t] matmuls (weights stationary); Q evicted as
    4-way block-diag qb4 (rmask multiply), K plain; V in natural [t,c]
    layout evicted into ones-augmented v33 tiles [t, (sl, j, c|1)] whose
    every 33rd column is a persistent 1.0 (memset once per buffer).
  - scores PRE-TRANSPOSED S^T[k,(j,q)]: causal -1000 mask added by a
    rank-128 const matmul (start of group), scores accumulate on top;
    exp on ACT gives exact zeros on masked entries.
  - P@V + rowsum FUSED: lhsT = pn block (P as stationary weights),
    rhs = [V_j | ones] 33 cols -> po2[q, (j, c|Z)] per slice. No separate
    zsum matmuls, no indicator broadcasts.
  - normalize: Z cols evicted, one reciprocal [128,32] per group, per-slice
    DVE multiply (po2 PSUM x broadcast recip) -> on[q, (hg, sl, j, c)] bf16.
  - one blocked DMA transpose per group -> on_t[(j,c), (hg, sl), q];
    out_proj bf16 (weights stationary), y evicted bf16, DRAM via SWDGE.
"""

import os
import sys

import numpy as np

sys.path.insert(0, "/opt/trn_rl_repo")

from contextlib import ExitStack

import concourse.bass as bass
import concourse.mybir as mybir
import concourse.tile as tile
from concourse import bacc
from concourse.bass_utils import run_bass_kernel_spmd

P = 128
NCORES = 8
NSLICE = 64  # slices per core
GSL = 4  # slices per group
NG = NSLICE // GSL  # groups per core
NH = 8
HD = 32
E = 256
S = 128

F32 = mybir.dt.float32
BF16 = mybir.dt.bfloat16
AX = mybir.AxisListType
ALU = mybir.AluOpType
AF = mybir.ActivationFunctionType


def build_program(ng=NG, repeats=1):
    nc = bacc.Bacc("TRN2", target_bir_lowering=False, debug=False, num_devices=NCORES)

    x_d = nc.dram_tensor("x", [ng, P, 2, GSL * S], BF16, kind="ExternalInput").ap()
    wq_d = nc.dram_tensor("wq", [P, 2, 256], BF16, kind="ExternalInput").ap()
    wk_d = nc.dram_tensor("wk", [P, 2, 256], BF16, kind="ExternalInput").ap()
    wv_d = nc.dram_tensor("wv", [P, 2, 256], BF16, kind="ExternalInput").ap()
    wo_d = nc.dram_tensor("wo", [P, 2, 256], BF16, kind="ExternalInput").ap()
    rm_d = nc.dram_tensor("rm2", [P, 4], F32, kind="ExternalInput").ap()
    am_d = nc.dram_tensor("amask", [P, S], BF16, kind="ExternalInput").ap()
    ni_d = nc.dram_tensor("negi", [P, 4 * S], BF16, kind="ExternalInput").ap()
    y_d = nc.dram_tensor("y", [ng, 2, P, GSL * S], BF16, kind="ExternalOutput").ap()

    with tile.TileContext(nc) as tc, ExitStack() as ctx:
        const = ctx.enter_context(tc.tile_pool(name="const", bufs=1))
        wq = const.tile([P, 2, 256], BF16, tag="wq")
        wk = const.tile([P, 2, 256], BF16, tag="wk")
        wv = const.tile([P, 2, 256], BF16, tag="wv")
        wo = const.tile([P, 2, 256], BF16, tag="wo")
        rm2 = const.tile([P, 4], F32, tag="rm2")
        amask = const.tile([P, S], BF16, tag="amask")
        negi = const.tile([P, 4 * S], BF16, tag="negi")
        v33a = const.tile([P, GSL, NH, 33], BF16, tag="v33a")
        v33b = const.tile([P, GSL, NH, 33], BF16, tag="v33b")
        nc.sync.dma_start(wq[:], wq_d)
        nc.sync.dma_start(wk[:], wk_d)
        nc.sync.dma_start(wv[:], wv_d)
        nc.sync.dma_start(wo[:], wo_d)
        nc.sync.dma_start(rm2[:], rm_d)
        nc.sync.dma_start(amask[:], am_d)
        nc.sync.dma_start(negi[:], ni_d)
        nc.vector.memset(v33a[:, :, :, 32], 1.0)
        nc.vector.memset(v33b[:, :, :, 32], 1.0)
        v33s = [v33a, v33b]

        xp = ctx.enter_context(tc.tile_pool(name="xp", bufs=3))
        qtp = ctx.enter_context(tc.tile_pool(name="qtp", bufs=2))
        ktp = ctx.enter_context(tc.tile_pool(name="ktp", bufs=2))
        pnp = ctx.enter_context(tc.tile_pool(name="pnp", bufs=4))
        rcpp = ctx.enter_context(tc.tile_pool(name="rcpp", bufs=2))
        onp = ctx.enter_context(tc.tile_pool(name="onp", bufs=2))
        otp = ctx.enter_context(tc.tile_pool(name="otp", bufs=2))
        ysp = ctx.enter_context(tc.tile_pool(name="ysp", bufs=2))

        # PSUM: 2 (scores) + 3 (in_proj + py) + 3 (po2) = 8 banks
        psS = ctx.enter_context(tc.tile_pool(name="psS", bufs=2, space="PSUM"))
        psD = ctx.enter_context(tc.tile_pool(name="psD", bufs=3, space="PSUM"))
        psO = ctx.enter_context(tc.tile_pool(name="psO", bufs=3, space="PSUM"))

        def emit_x_load(g):
            x = xp.tile([P, 2, GSL, S], BF16, tag="x")
            nc.sync.dma_start(x[:], x_d[g].rearrange("p c (sl s) -> p c sl s", sl=GSL))
            return x

        def emit_in_proj(x, g):
            """kt: [f', (hg, sl, t)] bf16; qb4: block-diag Q [f', (hg, u, sl, t)]
            bf16; v33: [t, (sl, j, c|1)] bf16 (ones col persists)."""
            qb4 = qtp.tile([P, 2, 4, GSL, S], BF16, tag="qb4")
            kt = ktp.tile([P, 2, GSL, S], BF16, tag="kt")
            v33 = v33s[g % 2]
            for which, wmat in ((0, wq), (1, wk)):
                for ft in range(2):
                    ps = psD.tile([P, GSL * S], F32, tag="d")
                    for ec in range(2):
                        nc.tensor.matmul(
                            ps[:],
                            lhsT=wmat[:, ec, ft * P : (ft + 1) * P],
                            rhs=x[:, ec].rearrange("p a b -> p (a b)"),
                            start=(ec == 0),
                            stop=(ec == 1),
                        )
                    if which == 0:
                        nc.vector.tensor_tensor(
                            qb4[:, ft].rearrange("p u a b -> p u (a b)"),
                            ps[:].rearrange("p (u f) -> p u f", u=1).broadcast_to(
                                [P, 4, GSL * S]
                            ),
                            rm2[:, :].rearrange("p (u f) -> p u f", f=1).broadcast_to(
                                [P, 4, GSL * S]
                            ),
                            ALU.mult,
                        )
                    else:
                        nc.scalar.copy(
                            kt[:, ft].rearrange("p a b -> p (a b)"), ps[:]
                        )
            for slp in range(GSL // 2):
                psv = psD.tile([P, 2, 256], F32, tag="d")
                for half in range(2):
                    sl = slp * 2 + half
                    for ec in range(2):
                        nc.tensor.matmul(
                            psv[:, half],
                            lhsT=x[:, ec, sl, :],
                            rhs=wv[:, ec, :],
                            start=(ec == 0),
                            stop=(ec == 1),
                        )
                dvv = v33[:, slp * 2 : slp * 2 + 2, :, 0:32]
                src = psv[:].rearrange("p a (b c) -> p a b c", b=NH)
                if slp == 0:
                    nc.scalar.copy(dvv, src)
                else:
                    nc.vector.tensor_copy(dvv, src)
            return qb4, kt, v33

        def emit_scores_exp(kqv, sl):
            """S^T[k,(j,q)] with causal -1000 via const mask matmul; exp on ACT
            (masked entries underflow to exact 0)."""
            qb4, kt, v33 = kqv
            pn = pnp.tile([P, 2, GSL, S], BF16, tag="pn")
            for hg in range(2):
                ssc = psS.tile([P, GSL * S], F32, tag="s")
                nc.tensor.matmul(
                    ssc[:],
                    lhsT=amask[:],
                    rhs=negi[:],
                    start=True,
                    stop=False,
                    skip_group_check=True,
                )
                nc.tensor.matmul(
                    ssc[:],
                    lhsT=kt[:, hg, sl, :],
                    rhs=qb4[:, hg, :, sl, :],
                    start=False,
                    stop=True,
                    skip_group_check=True,
                )
                nc.scalar.activation(
                    pn[:, hg].rearrange("p a b -> p (a b)"), ssc[:], AF.Exp
                )
            return pn

        def emit_pv2(kqv, pn, sl, zg):
            """po2[q, (j, c|Z)]: P stationary, rhs = [V_j | 1] (33 cols);
            Z column evicted into zg[:, sl]."""
            v33 = kqv[2]
            po2 = psO.tile([P, NH, 33], F32, tag="po2")
            for hg in range(2):
                for jp in range(4):
                    j = hg * 4 + jp
                    nc.tensor.matmul(
                        po2[:, j, :],
                        lhsT=pn[:, hg, jp, :],
                        rhs=v33[:, sl, j, :],
                        skip_group_check=True,
                    )
            nc.vector.tensor_copy(zg[:, sl], po2[:, :, 32])
            return po2

        def emit_norm_sl(po2, rcpz, on, sl):
            """on[q, (hg, sl, j, c)] = po2[q, (j, c)] * (1/Z) (broadcast)."""
            nc.vector.tensor_tensor(
                on[:, :, sl],
                po2[:, :, 0:32].rearrange("p (hg jp) c -> p hg jp c", hg=2),
                rcpz[:, sl].broadcast_to([P, 2, 4, 32]),
                ALU.mult,
            )

        def emit_out_proj(on_t, g):
            y_sb = ysp.tile([P, 2, GSL * S], BF16, tag="ysb")
            for et in range(2):
                py = psD.tile([P, GSL * S], F32, tag="d")
                for cc in range(2):
                    nc.tensor.matmul(
                        py[:],
                        lhsT=wo[:, cc, et * P : (et + 1) * P],
                        rhs=on_t[:, cc * GSL : (cc + 1) * GSL, :],
                        start=(cc == 0),
                        stop=(cc == 1),
                    )
                if et == 0:
                    nc.scalar.copy(y_sb[:, et], py[:])
                else:
                    nc.vector.tensor_copy(y_sb[:, et], py[:])
            nc.gpsimd.dma_start(y_d[g].rearrange("e p f -> p e f"), y_sb[:])

        def emit_recip_half(zg, rcpz, half):
            with nc.allow_low_precision(reason="softmax denominators"):
                nc.vector.reciprocal(
                    rcpz[:, 2 * half : 2 * half + 2].rearrange(
                        "p a b c d -> p (a b c d)"
                    ),
                    zg[:, 2 * half : 2 * half + 2].rearrange("p a b -> p (a b)"),
                )

        def emit_transpose(on):
            on_t = otp.tile([P, 2 * GSL, S], BF16, tag="ont")
            nc.sync.dma_start_transpose(
                on_t[:], on[:].rearrange("p a b c d -> p (a b c d)")
            )
            return on_t

        for _rep in range(repeats):
            xs = {}
            kqvs = {}
            pns = {}
            pending = None
            xs[0] = emit_x_load(0)
            kqvs[0] = emit_in_proj(xs[0], 0)
            for g in range(ng):
                kqv = kqvs[g]
                zg = rcpp.tile([P, GSL, NH], F32, tag="zg", name="zg")
                rcpz = rcpp.tile([P, GSL, 2, 4, 1], F32, tag="rcpz", name="rcpz")
                on = onp.tile([P, 2, GSL, 4, 32], BF16, tag="on", name="on")
                po2s = {}
                if g + 1 < ng:
                    xs[g + 1] = emit_x_load(g + 1)
                if (g, 0) not in pns:
                    pns[(g, 0)] = emit_scores_exp(kqv, 0)
                pns[(g, 1)] = emit_scores_exp(kqv, 1)
                po2s[0] = emit_pv2(kqv, pns[(g, 0)], 0, zg)
                if pending is not None:
                    emit_out_proj(*pending)
                    pending = None
                if g + 1 < ng:
                    kqvs[g + 1] = emit_in_proj(xs[g + 1], g + 1)
                pns[(g, 2)] = emit_scores_exp(kqv, 2)
                po2s[1] = emit_pv2(kqv, pns[(g, 1)], 1, zg)
                emit_recip_half(zg, rcpz, 0)
                emit_norm_sl(po2s[0], rcpz, on, 0)
                emit_norm_sl(po2s[1], rcpz, on, 1)
                pns[(g, 3)] = emit_scores_exp(kqv, 3)
                po2s[2] = emit_pv2(kqv, pns[(g, 2)], 2, zg)
                po2s[3] = emit_pv2(kqv, pns[(g, 3)], 3, zg)
                if g + 1 < ng:
                    pns[(g + 1, 0)] = emit_scores_exp(kqvs[g + 1], 0)
                emit_recip_half(zg, rcpz, 1)
                emit_norm_sl(po2s[2], rcpz, on, 2)
                emit_norm_sl(po2s[3], rcpz, on, 3)
                pending = (emit_transpose(on), g)
                for sl in range(GSL):
                    del pns[(g, sl)]
                del kqvs[g], xs[g]
            emit_out_proj(*pending)

    nc.compile()
    return nc


def prep_inputs(hidden_state, w_in, w_out):
    """Host-side prep: permute weights per-head, transpose x, shard."""
    import ml_dtypes

    bf16 = ml_dtypes.bfloat16
    B, S_, H, W, E_ = hidden_state.shape
    nsl = B * H * W
    scale = 1.0 / np.sqrt(HD)

    idx_q = np.concatenate([np.arange(i * 96, i * 96 + 32) for i in range(NH)])
    idx_k = idx_q + 32
    idx_v = idx_q + 64
    Wq = (w_in[idx_q] * scale).astype(np.float32)  # [256 f, 256 e]
    Wk = w_in[idx_k].astype(np.float32)
    Wv = w_in[idx_v].astype(np.float32)

    def pack_w(Wm):
        # [p, ec, f] with w[p, ec, f] = Wm[f, ec*128+p]
        return np.ascontiguousarray(
            Wm.T.reshape(2, P, 256).transpose(1, 0, 2)
        ).astype(bf16)

    wq_h = pack_w(Wq)
    wk_h = pack_w(Wk)
    wv_h = pack_w(Wv)
    wo_h = pack_w(w_out.astype(np.float32))

    # rm2[p, u] = 1 iff p//32 == u (block-diag eviction masks)
    rm2_h = np.zeros((P, 4), np.float32)
    for p in range(P):
        rm2_h[p, p // 32] = 1.0

    # mask matmul constants (S^T orientation): out[k,(j,q)] = -1000*am[q,k]
    #   want -1000 iff k > q  ->  am = triu(ones, +1)
    am_h = np.triu(np.ones((S, S), np.float32), 1).astype(bf16)
    ni_h = np.ascontiguousarray(
        (-1000.0 * np.eye(S, dtype=np.float32))[:, None, :].repeat(4, 1).reshape(S, 4 * S)
    ).astype(bf16)

    # x^T per slice: [slice, e, s]
    xt = hidden_state.transpose(0, 2, 3, 4, 1).reshape(nsl, E_, S_).astype(bf16)

    in_maps = []
    for c in range(NCORES):
        xs = xt[c * NSLICE : (c + 1) * NSLICE]  # [64, 256, 128]
        xs = xs.reshape(NG, GSL, 2, P, S_).transpose(0, 3, 2, 1, 4)
        xs = np.ascontiguousarray(xs.reshape(NG, P, 2, GSL * S_))
        in_maps.append(
            {
                "x": xs,
                "wq": wq_h,
                "wk": wk_h,
                "wv": wv_h,
                "wo": wo_h,
                "rm2": rm2_h,
                "amask": am_h,
                "negi": ni_h,
            }
        )
    return in_maps


def assemble_output(results, B=2, H=16, W=16):
    """results: list of 8 dicts with 'y' [NG, 2, 128, GSL*S] bf16."""
    ys = []
    for c in range(NCORES):
        y = np.asarray(results[c]["y"], dtype=np.float32)
        y = y.reshape(NG, 2, P, GSL, S)
        y = y.transpose(0, 3, 1, 2, 4).reshape(NSLICE, E, S)
        ys.append(y)
    y_all = np.concatenate(ys, axis=0)  # [512, 256 e, 128 s]
    y_all = y_all.transpose(0, 2, 1)  # [512, s, e]
    out = y_all.reshape(B, H, W, S, E).transpose(0, 3, 1, 2, 4)
    return np.ascontiguousarray(out.astype(np.float32))


_NC_CACHE = {}


def get_program(repeats=1):
    key = repeats
    if key not in _NC_CACHE:
        _NC_CACHE[key] = build_program(repeats=repeats)
    return _NC_CACHE[key]


class _Executor:
    """Cached PJRT executor: builds the shard_map jit once, reuses across calls."""

    def __init__(self, nc):
        import jax
        from jax.sharding import Mesh, PartitionSpec
        from jax.experimental.shard_map import shard_map
        from concourse.bass2jax import _bass_exec_p, install_neuronx_cc_hook, partition_id_tensor

        install_neuronx_cc_hook()
        self.nc = nc
        pname = nc.partition_id_tensor.name if nc.partition_id_tensor else None
        in_names, out_names, out_avals, zero_outs = [], [], [], []
        for alloc in nc.m.functions[0].allocations:
            if not isinstance(alloc, mybir.MemoryLocationSet):
                continue
            name = alloc.memorylocations[0].name
            if alloc.kind == "ExternalInput":
                if name != pname:
                    in_names.append(name)
            elif alloc.kind == "ExternalOutput":
                out_names.append(name)
                shape = tuple(alloc.tensor_shape)
                dtype = mybir.dt.np(alloc.dtype)
                out_avals.append(jax.core.ShapedArray(shape, dtype))
                zero_outs.append(np.zeros(shape, dtype))
        self.in_names = in_names
        self.out_names = out_names
        self.out_avals = out_avals
        n_params = len(in_names)
        all_names = in_names + out_names + ([pname] if pname else [])

        def _body(*args):
            operands = list(args)
            if pname is not None:
                operands.append(partition_id_tensor())
            return tuple(
                _bass_exec_p.bind(
                    *operands,
                    out_avals=tuple(out_avals),
                    in_names=tuple(all_names),
                    out_names=tuple(out_names),
                    lowering_input_output_aliases=(),
                    sim_require_finite=True,
                    sim_require_nnan=True,
                    nc=nc,
                )
            )

        devices = jax.devices()[:NCORES]
        mesh = Mesh(np.asarray(devices), ("core",))
        n_outs = len(out_avals)
        self._jit = jax.jit(
            shard_map(
                _body,
                mesh=mesh,
                in_specs=(PartitionSpec("core"),) * (n_params + n_outs),
                out_specs=(PartitionSpec("core"),) * n_outs,
                check_rep=False,
            ),
            keep_unused=True,
        )
        self._zero_concat = [
            np.zeros((NCORES * z.shape[0], *z.shape[1:]), z.dtype) for z in zero_outs
        ]
        self._jax = jax

    def run(self, in_maps):
        concat_in = [
            np.concatenate([np.asarray(in_maps[c][nm]) for c in range(NCORES)], axis=0)
            for nm in self.in_names
        ]
        outs = self._jit(*concat_in, *self._zero_concat)
        self._jax.block_until_ready(outs)
        return [
            {
                nm: np.asarray(outs[i]).reshape(NCORES, *self.out_avals[i].shape)[c]
                for i, nm in enumerate(self.out_names)
            }
            for c in range(NCORES)
        ]


_EXEC_CACHE = {}


def get_executor(repeats=1):
    if repeats not in _EXEC_CACHE:
        _EXEC_CACHE[repeats] = _Executor(get_program(repeats))
    return _EXEC_CACHE[repeats]


def kernel(hidden_state, w_in, w_out, repeats=1):
    hidden_state = np.asarray(hidden_state, dtype=np.float32)
    w_in = np.asarray(w_in, dtype=np.float32)
    w_out = np.asarray(w_out, dtype=np.float32)
    ex = get_executor(repeats)
    in_maps = prep_inputs(hidden_state, w_in, w_out)
    results = ex.run(in_maps)
    return assemble_output(results)
